# revision 25
# baseline (speedup 1.0000x reference)
"""ClusterMI kernel for 8 Trainium2 NeuronCores.

Row-sharded KNN-MI:
- Host: normalize X rows (fp32, matching reference), transpose to [D=128, N]
  fp32. Sort rows by class; pad each class to a multiple of 128 rows so each
  core owns 17 class-pure 128-row blocks (8*17*128 = 17408 padded rows).
  Columns stay in ORIGINAL order (16384 wide, no padding).
- Device (per core, SPMD), per block b (interleaved phases, lhsT stationary
  across the whole block):
  Anchors: matmul block rows against a pre-gathered zero-padded [D, CLS_W]
  slab of the block's class (fp32r: full-rate PE, ~2^-12 relative noise
  vs bf16's 2^-8, keeping threshold count flips ~zero) -> PSUM fp32 dots in
  two <=1024 chunks; DVE max8 on each chunk directly from PSUM + combining
  max8 -> top-8 same-class dots; the 4th largest is the anchor threshold t
  (dot of the K=3rd same-class neighbor; d = 1 - dot is monotone decreasing
  in dot; self-dot ~1 occupies rank 1). The xc slabs live in a 6-deep
  rotating SBUF window (fp32 slabs are too big to keep all 17 resident).
  Counts: stream all 16384 columns in 1024-wide strips; per strip one engine
  counts dots >= t: DVE tensor_scalar(is_ge, add, accum_out) [exact] or ACT
  activation(Sign, scale=-1, bias=t, accum_out) [sign-sum; host converts;
  anchor-equality half-count fixed by ceil]. Strips are split DVE:ACT =
  124:148 per engine cycle costs (DVE 0.96 GHz + PSUM access penalty vs
  ACT 1.2 GHz).
- Host: m_i = (count >= t) - 1 (self), digamma in float64, assemble the
  scalar exactly like the reference formula.

Built as a Bacc module: bacc.compile() runs generate_event_semaphores which
legalizes multi-wait instructions for CoreV3 walrus (at most 1 sync wait per
instruction), so no manual observer/prime tricks are needed.
"""

import sys
import numpy as np

sys.path.insert(0, "/opt/trn_rl_repo")

N = 16384
D = 128
C = 10
K = 3
EPS = 1e-8
NCORES = 8
BLOCKS = 17                      # 128-row blocks per core
RPC = BLOCKS * 128               # rows per core (padded)
NPAD = NCORES * RPC              # 17408 padded rows
STRIP = 1024                     # count strip width
NSTRIPS = N // STRIP             # 16 strips per block
XC_BUFS = 6                      # rotating xc slab window depth
XC_EARLY = 6                     # slabs DMAd from host; the rest are gathered
                                 # on-device from sb_xt by GpSimd (saves HBM)

_CACHE = {}


N_DVE_TILES = 119   # DVE tensor_scalar counts; ACT takes the rest


def _phase_b_schedule():
    """Deterministic DVE/ACT assignment for the 17*16 count tiles.

    Measured per-tile HW costs: DVE 1430 ns (tensor_scalar + accumulator
    read + PSUM access), ACT 1382 ns (activate + accumulator read). DVE also
    carries the 17 per-block max8 chains (~2.4 us each):
    solve 40.2us + 1430*x = 1382*(272-x) -> x = 119.
    """
    total = BLOCKS * NSTRIPS
    n_dve = N_DVE_TILES
    sched = []
    acc = 0
    for _ in range(total):
        acc += n_dve
        if acc >= total:
            acc -= total
            sched.append("D")
        else:
            sched.append("A")
    assert sched.count("D") == n_dve
    return sched


def _build_program(cls_w):
    import concourse.tile as tile
    from concourse import mybir
    from concourse.bacc import Bacc
    from contextlib import ExitStack

    F32 = mybir.dt.float32
    F32R = mybir.dt.float32r
    U16 = mybir.dt.uint16
    AF = mybir.ActivationFunctionType
    ALU = mybir.AluOpType

    assert cls_w % 128 == 0 and STRIP < cls_w <= 2 * STRIP
    iw = cls_w // 16              # wrapped index columns per slab

    nc = Bacc("TRN2", target_bir_lowering=False)
    # xt carries 16 zero pad columns at the end: gather pad-indices point there
    xt = nc.dram_tensor("xt", [D, N + 16], F32R, kind="ExternalInput")
    xr = nc.dram_tensor("xr", [D, RPC], F32R, kind="ExternalInput")
    xc = nc.dram_tensor("xc", [D, XC_EARLY * cls_w], F32R, kind="ExternalInput")
    xi = nc.dram_tensor("xi", [128, (BLOCKS - XC_EARLY) * iw], U16,
                        kind="ExternalInput")
    out_t = nc.dram_tensor("out_t", [128, BLOCKS * 8], F32, kind="ExternalOutput")
    out_d = nc.dram_tensor("out_d", [128, BLOCKS * NSTRIPS], F32, kind="ExternalOutput")
    out_a = nc.dram_tensor("out_a", [128, BLOCKS * NSTRIPS], F32, kind="ExternalOutput")

    sched = _phase_b_schedule()
    w1 = STRIP                # first anchor chunk width
    w2 = cls_w - STRIP        # second anchor chunk width (<= 1024)

    with tile.TileContext(nc) as tc:
        with ExitStack() as ctx:
            singles = ctx.enter_context(tc.tile_pool(name="singles", bufs=1))
            xcp = ctx.enter_context(tc.tile_pool(name="xcp", bufs=XC_BUFS))
            gtmp = ctx.enter_context(tc.tile_pool(name="gtmp", bufs=2))
            ps = ctx.enter_context(tc.tile_pool(name="ps", bufs=4, space="PSUM"))

            sb_xr = singles.tile([D, RPC], F32R)
            nc.sync.dma_start(out=sb_xr, in_=xr[:, :])
            sb_xt = singles.tile([D, N + 16], F32R)
            nc.sync.dma_start(out=sb_xt[:, N:N + 16], in_=xt[:, N:N + 16])
            sb_xi = singles.tile([128, (BLOCKS - XC_EARLY) * iw], U16)
            nc.sync.dma_start(out=sb_xi, in_=xi[:, :])

            # Interleave xc slab / xt strip DMAs so early blocks' slabs and
            # early strips arrive first; slabs rotate through XC_BUFS slots.
            # Slabs for blocks >= XC_EARLY are gathered from sb_xt on GpSimd
            # once all xt strips have landed (emitted lazily below).
            dma_slabs = []
            xt_loaded = 0
            for b in range(XC_EARLY):
                t_xc = xcp.tile([D, cls_w], F32R, tag="xc")
                s = b * cls_w
                h = cls_w // 2
                nc.sync.dma_start(out=t_xc[:, 0:h], in_=xc[:, s:s + h])
                nc.sync.dma_start(out=t_xc[:, h:cls_w], in_=xc[:, s + h:s + cls_w])
                dma_slabs.append(t_xc)
                if xt_loaded < NSTRIPS:
                    for c in range(xt_loaded, min(xt_loaded + 3, NSTRIPS)):
                        cs = c * STRIP
                        nc.sync.dma_start(out=sb_xt[:, cs:cs + STRIP],
                                          in_=xt[:, cs:cs + STRIP])
                    xt_loaded = min(xt_loaded + 3, NSTRIPS)
            for c in range(xt_loaded, NSTRIPS):
                cs = c * STRIP
                nc.sync.dma_start(out=sb_xt[:, cs:cs + STRIP],
                                  in_=xt[:, cs:cs + STRIP])

            xc_tiles = {}
            for b in range(XC_EARLY):
                xc_tiles[b] = dma_slabs[b]

            def make_slab(b):
                # gather block b's class columns from sb_xt (GpSimd), then
                # bounce SBUF->SBUF through DMA: fp32r matmuls only accept
                # DMA-produced (or pre-rounded) inputs.
                tmp = gtmp.tile([D, cls_w], F32, tag="gt")
                ib = (b - XC_EARLY) * iw
                for gs in range(0, cls_w, STRIP):
                    gw = min(STRIP, cls_w - gs)
                    nc.gpsimd.indirect_copy(
                        tmp[:, gs:gs + gw],
                        sb_xt[:, 0:N + 16].bitcast(F32),
                        sb_xi[:, ib + gs // 16: ib + (gs + gw) // 16], True)
                t_xc = xcp.tile([D, cls_w], F32R, tag="xc")
                nc.sync.dma_start(out=t_xc, in_=tmp.bitcast(F32R))
                xc_tiles[b] = t_xc

            sb_t8 = singles.tile([128, BLOCKS * 8], F32)
            sb_m16 = singles.tile([128, BLOCKS * 16], F32)
            sb_cd = singles.tile([128, BLOCKS * NSTRIPS], F32)
            sb_ca = singles.tile([128, BLOCKS * NSTRIPS], F32)
            nc.gpsimd.memset(sb_cd, 0.0)
            nc.gpsimd.memset(sb_ca, 0.0)

            def anchors_p1(b):
                # same-class dots chunk 1 -> top-8
                lhsT = sb_xr[:, b * 128:(b + 1) * 128]
                slab = xc_tiles[b]
                pa1 = ps.tile([128, STRIP], F32, tag="pp")
                for w in range(0, w1, 512):
                    cw = min(512, w1 - w)
                    nc.tensor.matmul(pa1[:, w:w + cw], lhsT, slab[:, w:w + cw],
                                     start=True, stop=True)
                m16 = sb_m16[:, b * 16:(b + 1) * 16]
                nc.vector.max(out=m16[:, 0:8], in_=pa1)

            def anchors_p2(b):
                # same-class dots chunk 2 -> top-8
                lhsT = sb_xr[:, b * 128:(b + 1) * 128]
                slab = xc_tiles[b]
                pa2 = ps.tile([128, STRIP], F32, tag="pp")
                for w in range(0, w2, 512):
                    cw = min(512, w2 - w)
                    nc.tensor.matmul(pa2[:, w:w + cw], lhsT,
                                     slab[:, w1 + w:w1 + w + cw],
                                     start=True, stop=True)
                m16 = sb_m16[:, b * 16:(b + 1) * 16]
                nc.vector.max(out=m16[:, 8:16], in_=pa2[:, 0:w2])

            def anchors_p3(b):
                # combine the two chunk top-8s -> threshold t8[b*8+3]
                m16 = sb_m16[:, b * 16:(b + 1) * 16]
                nc.vector.max(out=sb_t8[:, b * 8:(b + 1) * 8], in_=m16)

            def count_tile(b, c):
                # threshold count over strip c for block b
                lhsT = sb_xr[:, b * 128:(b + 1) * 128]
                t_ap = sb_t8[:, b * 8 + 3: b * 8 + 4]
                cs = c * STRIP
                slot = b * NSTRIPS + c
                pt = ps.tile([128, STRIP], F32, tag="pp")
                for w in range(0, STRIP, 512):
                    nc.tensor.matmul(pt[:, w:w + 512], lhsT,
                                     sb_xt[:, cs + w:cs + w + 512],
                                     start=True, stop=True)
                if sched[slot] == "D":
                    nc.vector.tensor_scalar(pt, pt, t_ap, None,
                                            op0=ALU.is_ge, op1=ALU.add,
                                            accum_out=sb_cd[:, slot:slot + 1])
                else:
                    nc.scalar.activation(out=pt, in_=pt, func=AF.Sign,
                                         bias=t_ap, scale=-1.0,
                                         accum_out=sb_ca[:, slot:slot + 1])

            # Anchors run 2 blocks ahead of counts so the max8->threshold
            # chain never gates the count engines; anchor parts are spread
            # between count tiles so the max8 ops never form a burst in the
            # DVE queue.
            def block_step(b):
                for c in range(NSTRIPS):
                    if c == 1 and b + 2 < BLOCKS and b + 2 >= XC_EARLY:
                        make_slab(b + 2)
                    elif c == 3 and b + 2 < BLOCKS:
                        anchors_p1(b + 2)
                    elif c == 9 and b + 2 < BLOCKS:
                        anchors_p2(b + 2)
                    elif c == 13 and b + 2 < BLOCKS:
                        anchors_p3(b + 2)
                    count_tile(b, c)

            anchors_p1(0)
            anchors_p2(0)
            anchors_p3(0)
            anchors_p1(1)
            anchors_p2(1)
            anchors_p3(1)
            for b in range(BLOCKS):
                block_step(b)

            nc.sync.dma_start(out=out_t[:, :], in_=sb_t8)
            nc.sync.dma_start(out=out_d[:, :], in_=sb_cd)
            nc.sync.dma_start(out=out_a[:, :], in_=sb_ca)

    nc.compile()
    return nc


def _digamma(x):
    x = np.asarray(x, np.float64).copy()
    res = np.zeros_like(x)
    for _ in range(6):
        small = x < 6.0
        if not small.any():
            break
        res[small] -= 1.0 / x[small]
        x[small] += 1.0
    inv = 1.0 / x
    inv2 = inv * inv
    res += (np.log(x) - 0.5 * inv
            - inv2 * (1.0 / 12 - inv2 * (1.0 / 120 - inv2 * (1.0 / 252 - inv2 / 240))))
    return res


def _prep(X, y):
    """Host-side normalize/sort/pad. Returns per-core input maps + row map."""
    X = np.asarray(X, np.float32)
    y = np.asarray(y).astype(np.int64)
    norms = np.maximum(np.linalg.norm(X, axis=1, keepdims=True), np.float32(EPS))
    Xn = (X / norms).astype(np.float32)
    XnT = np.ascontiguousarray(Xn.T)  # [D, N] fp32

    XnT_pad = np.zeros((D, N + 16), np.float32)
    XnT_pad[:, :N] = XnT
    counts = np.bincount(y, minlength=C)
    cls_w = max(STRIP + 128, int(-(-counts.max() // 128) * 128))
    members = [np.nonzero(y == c)[0] for c in range(C)]

    # padded sorted row layout: class-pure 128-blocks
    row_orig = np.full(NPAD, -1, np.int64)
    blk_cls = np.full(NPAD // 128, -1, np.int64)
    pos = 0
    for c in range(C):
        n_c = counts[c]
        row_orig[pos:pos + n_c] = members[c]
        nblk = -(-n_c // 128)
        blk_cls[pos // 128: pos // 128 + nblk] = c
        pos += nblk * 128
    assert pos <= NPAD

    zcol = np.zeros((D, 1), np.float32)
    iw = cls_w // 16
    in_maps = []
    for k in range(NCORES):
        rows = row_orig[k * RPC:(k + 1) * RPC]
        xr = np.where(rows[None, :] >= 0, XnT[:, np.maximum(rows, 0)], zcol)
        xr = np.ascontiguousarray(xr, dtype=np.float32)
        xc = np.zeros((D, XC_EARLY * cls_w), np.float32)
        xi = np.zeros((128, (BLOCKS - XC_EARLY) * iw), np.uint16)
        for b in range(BLOCKS):
            c = blk_cls[(k * RPC) // 128 + b]
            mem = members[c] if c >= 0 else np.empty(0, np.int64)
            if b < XC_EARLY:
                xc[:, b * cls_w: b * cls_w + len(mem)] = XnT[:, mem]
            else:
                # wrapped uint16 indices; pad points at the zero columns at N
                ids = np.full(cls_w, N, np.uint16)
                ids[:len(mem)] = mem.astype(np.uint16)
                wrap = np.zeros((16, iw), np.uint16)
                pos = np.arange(cls_w)
                wrap[pos % 16, pos // 16] = ids
                xi[:, (b - XC_EARLY) * iw:(b - XC_EARLY + 1) * iw] = np.tile(wrap, (8, 1))
        in_maps.append({"xt": XnT_pad, "xr": xr, "xc": xc, "xi": xi})
    return in_maps, row_orig, counts, cls_w


def _m_counts_cpu(X, y):
    """Exact fp32 replication of the reference's m_i computation, blocked."""
    X = np.asarray(X, np.float32)
    y = np.asarray(y).astype(np.int64)
    norms = np.maximum(np.linalg.norm(X, axis=1, keepdims=True), np.float32(EPS))
    Xn = (X / norms).astype(np.float32)
    m = np.empty(N, np.int64)
    B = 2048
    for s in range(0, N, B):
        e = s + B
        d = (np.float32(1.0) - Xn[s:e] @ Xn.T).astype(np.float32)
        idx = np.arange(s, e)
        d[np.arange(e - s), idx] = 0.0
        ds = np.where(y[s:e, None] == y[None, :], d, np.float32(1.0e7))
        anchor = np.partition(ds, K, axis=1)[:, K]
        m[s:e] = (d <= anchor[:, None]).sum(axis=1) - 1
    return m, np.bincount(y, minlength=C)


def _combine_counts(results, row_orig):
    """Device outputs -> per-point neighbor counts m."""
    sched = _phase_b_schedule()
    is_act = np.array([s == "A" for s in sched])
    nact = is_act.reshape(BLOCKS, NSTRIPS).sum(axis=1)

    m = np.full(N, -1, np.int64)
    for k in range(NCORES):
        o_d = np.asarray(results[k]["out_d"], np.float64)
        o_a = np.asarray(results[k]["out_a"], np.float64)
        for b in range(BLOCKS):
            rows = row_orig[k * RPC + b * 128: k * RPC + (b + 1) * 128]
            sl = slice(b * NSTRIPS, (b + 1) * NSTRIPS)
            tot = (o_d[:, sl].sum(axis=1)
                   + (nact[b] * STRIP - o_a[:, sl].sum(axis=1)) / 2.0)
            cnt = np.ceil(tot - 1e-9).astype(np.int64)  # anchor half-count -> up
            valid = rows >= 0
            m[rows[valid]] = cnt[valid] - 1  # minus self
    assert (m >= 1).all(), f"bad counts: min {m.min()}"
    return m


def _m_counts_device(X, y):
    from concourse.bass_utils import run_bass_kernel_spmd

    in_maps, row_orig, counts, cls_w = _prep(X, y)
    key = ("prog", cls_w)
    if key not in _CACHE:
        _CACHE[key] = _build_program(cls_w)
    res = run_bass_kernel_spmd(_CACHE[key], in_maps, core_ids=list(range(NCORES)))
    return _combine_counts(res.results, row_orig), counts


def kernel(X, y):
    import os
    m = None
    if os.environ.get("CLUSTERMI_SKIP_DEVICE") != "1":
        try:
            m, counts = _m_counts_device(X, y)
        except Exception as e:
            if os.environ.get("CLUSTERMI_NO_FALLBACK") == "1":
                raise
            sys.stderr.write(f"clustermi: device path failed ({e}); CPU fallback\n")
    if m is None:
        m, counts = _m_counts_cpu(X, y)

    avg_m = _digamma(m.astype(np.float64)).mean()
    counts_f = counts.astype(np.float64)
    avg_N_x = float((counts_f / N * _digamma(counts_f)).sum())
    mi = float(_digamma(np.float64(N)) - avg_N_x + _digamma(np.float64(K)) - avg_m)
    return np.float32(mi / np.log(2.0))


# revision 28
# speedup vs baseline: 2.2737x; 2.2737x over previous
"""ClusterMI kernel for 8 Trainium2 NeuronCores.

Row-sharded KNN-MI:
- Host: normalize X rows (fp32, matching reference), transpose to [D=128, N]
  fp32. Sort rows by class; pad each class to a multiple of 128 rows so each
  core owns 17 class-pure 128-row blocks (8*17*128 = 17408 padded rows).
  Columns stay in ORIGINAL order (16384 wide, no padding).
- Device (per core, SPMD), per block b (interleaved phases, lhsT stationary
  across the whole block):
  Anchors: matmul block rows against a pre-gathered zero-padded [D, CLS_W]
  slab of the block's class (fp32r: full-rate PE, ~2^-12 relative noise
  vs bf16's 2^-8, keeping threshold count flips ~zero) -> PSUM fp32 dots in
  two <=1024 chunks; DVE max8 on each chunk directly from PSUM + combining
  max8 -> top-8 same-class dots; the 4th largest is the anchor threshold t
  (dot of the K=3rd same-class neighbor; d = 1 - dot is monotone decreasing
  in dot; self-dot ~1 occupies rank 1). The xc slabs live in a 6-deep
  rotating SBUF window (fp32 slabs are too big to keep all 17 resident).
  Counts: stream all 16384 columns in 1024-wide strips; per strip one engine
  counts dots >= t: DVE tensor_scalar(is_ge, add, accum_out) [exact] or ACT
  activation(Sign, scale=-1, bias=t, accum_out) [sign-sum; host converts;
  anchor-equality half-count fixed by ceil]. Strips are split DVE:ACT =
  124:148 per engine cycle costs (DVE 0.96 GHz + PSUM access penalty vs
  ACT 1.2 GHz).
- Host: m_i = (count >= t) - 1 (self), digamma in float64, assemble the
  scalar exactly like the reference formula.

Built as a Bacc module: bacc.compile() runs generate_event_semaphores which
legalizes multi-wait instructions for CoreV3 walrus (at most 1 sync wait per
instruction), so no manual observer/prime tricks are needed.
"""

import sys
import numpy as np

sys.path.insert(0, "/opt/trn_rl_repo")

N = 16384
D = 128
C = 10
K = 3
EPS = 1e-8
NCORES = 8
BLOCKS = 17                      # 128-row blocks per core
RPC = BLOCKS * 128               # rows per core (padded)
NPAD = NCORES * RPC              # 17408 padded rows
STRIP = 1024                     # count strip width
NSTRIPS = N // STRIP             # 16 strips per block
XC_BUFS = 6                      # rotating xc slab window depth
XC_EARLY = BLOCKS               # all slabs DMAd from host (on-device GpSimd
                                 # gather measured 20x slower than DMA)

_CACHE = {}


N_DVE_TILES = 119   # DVE tensor_scalar counts; ACT takes the rest


def _phase_b_schedule():
    """Deterministic DVE/ACT assignment for the 17*16 count tiles.

    Measured per-tile HW costs: DVE 1430 ns (tensor_scalar + accumulator
    read + PSUM access), ACT 1382 ns (activate + accumulator read). DVE also
    carries the 17 per-block max8 chains (~2.4 us each):
    solve 40.2us + 1430*x = 1382*(272-x) -> x = 119.
    """
    total = BLOCKS * NSTRIPS
    n_dve = N_DVE_TILES
    sched = []
    acc = 0
    for _ in range(total):
        acc += n_dve
        if acc >= total:
            acc -= total
            sched.append("D")
        else:
            sched.append("A")
    assert sched.count("D") == n_dve
    return sched


def _build_program(cls_w):
    import concourse.tile as tile
    from concourse import mybir
    from concourse.bacc import Bacc
    from contextlib import ExitStack

    F32 = mybir.dt.float32
    F32R = mybir.dt.float32r
    U16 = mybir.dt.uint16
    AF = mybir.ActivationFunctionType
    ALU = mybir.AluOpType

    assert cls_w % 128 == 0 and STRIP < cls_w <= 2 * STRIP
    iw = cls_w // 16              # wrapped index columns per slab

    nc = Bacc("TRN2", target_bir_lowering=False)
    # xt carries 16 zero pad columns at the end: gather pad-indices point there
    xt = nc.dram_tensor("xt", [D, N + 16], F32R, kind="ExternalInput")
    xr = nc.dram_tensor("xr", [D, RPC], F32R, kind="ExternalInput")
    xc = nc.dram_tensor("xc", [D, XC_EARLY * cls_w], F32R, kind="ExternalInput")
    xi = (nc.dram_tensor("xi", [128, (BLOCKS - XC_EARLY) * iw], U16,
                         kind="ExternalInput")
          if BLOCKS > XC_EARLY else None)
    out_t = nc.dram_tensor("out_t", [128, BLOCKS * 8], F32, kind="ExternalOutput")
    out_d = nc.dram_tensor("out_d", [128, BLOCKS * NSTRIPS], F32, kind="ExternalOutput")
    out_a = nc.dram_tensor("out_a", [128, BLOCKS * NSTRIPS], F32, kind="ExternalOutput")

    sched = _phase_b_schedule()
    w1 = STRIP                # first anchor chunk width
    w2 = cls_w - STRIP        # second anchor chunk width (<= 1024)

    with tile.TileContext(nc) as tc:
        with ExitStack() as ctx:
            singles = ctx.enter_context(tc.tile_pool(name="singles", bufs=1))
            xcp = ctx.enter_context(tc.tile_pool(name="xcp", bufs=XC_BUFS))
            gtmp = ctx.enter_context(tc.tile_pool(name="gtmp", bufs=2))
            ps = ctx.enter_context(tc.tile_pool(name="ps", bufs=4, space="PSUM"))

            sb_xr = singles.tile([D, RPC], F32R)
            nc.sync.dma_start(out=sb_xr, in_=xr[:, :])
            sb_xt = singles.tile([D, N + 16], F32R)
            nc.sync.dma_start(out=sb_xt[:, N:N + 16], in_=xt[:, N:N + 16])
            if xi is not None:
                sb_xi = singles.tile([128, (BLOCKS - XC_EARLY) * iw], U16)
                nc.sync.dma_start(out=sb_xi, in_=xi[:, :])

            # Interleave xc slab / xt strip DMAs so early blocks' slabs and
            # early strips arrive first; slabs rotate through XC_BUFS slots.
            # Slabs for blocks >= XC_EARLY are gathered from sb_xt on GpSimd
            # once all xt strips have landed (emitted lazily below).
            dma_slabs = []
            xt_loaded = 0
            for b in range(XC_EARLY):
                t_xc = xcp.tile([D, cls_w], F32R, tag="xc")
                s = b * cls_w
                h = cls_w // 2
                nc.sync.dma_start(out=t_xc[:, 0:h], in_=xc[:, s:s + h])
                nc.sync.dma_start(out=t_xc[:, h:cls_w], in_=xc[:, s + h:s + cls_w])
                dma_slabs.append(t_xc)
                if xt_loaded < NSTRIPS:
                    for c in range(xt_loaded, min(xt_loaded + 3, NSTRIPS)):
                        cs = c * STRIP
                        nc.sync.dma_start(out=sb_xt[:, cs:cs + STRIP],
                                          in_=xt[:, cs:cs + STRIP])
                    xt_loaded = min(xt_loaded + 3, NSTRIPS)
            for c in range(xt_loaded, NSTRIPS):
                cs = c * STRIP
                nc.sync.dma_start(out=sb_xt[:, cs:cs + STRIP],
                                  in_=xt[:, cs:cs + STRIP])

            xc_tiles = {}
            for b in range(XC_EARLY):
                xc_tiles[b] = dma_slabs[b]

            def make_slab(b):
                # gather block b's class columns from sb_xt (GpSimd), then
                # bounce SBUF->SBUF through DMA: fp32r matmuls only accept
                # DMA-produced (or pre-rounded) inputs.
                tmp = gtmp.tile([D, cls_w], F32, tag="gt")
                ib = (b - XC_EARLY) * iw
                for gs in range(0, cls_w, STRIP):
                    gw = min(STRIP, cls_w - gs)
                    nc.gpsimd.indirect_copy(
                        tmp[:, gs:gs + gw],
                        sb_xt[:, 0:N + 16].bitcast(F32),
                        sb_xi[:, ib + gs // 16: ib + (gs + gw) // 16], True)
                t_xc = xcp.tile([D, cls_w], F32R, tag="xc")
                nc.sync.dma_start(out=t_xc, in_=tmp.bitcast(F32R))
                xc_tiles[b] = t_xc

            sb_t8 = singles.tile([128, BLOCKS * 8], F32)
            sb_m16 = singles.tile([128, BLOCKS * 16], F32)
            sb_cd = singles.tile([128, BLOCKS * NSTRIPS], F32)
            sb_ca = singles.tile([128, BLOCKS * NSTRIPS], F32)
            nc.gpsimd.memset(sb_cd, 0.0)
            nc.gpsimd.memset(sb_ca, 0.0)

            anchor_tiles = {}

            def anchors_mm1(b):
                # same-class dots chunk 1 (PE only)
                lhsT = sb_xr[:, b * 128:(b + 1) * 128]
                slab = xc_tiles[b]
                pa1 = ps.tile([128, STRIP], F32, tag="pp")
                for w in range(0, w1, 512):
                    cw = min(512, w1 - w)
                    nc.tensor.matmul(pa1[:, w:w + cw], lhsT, slab[:, w:w + cw],
                                     start=True, stop=True)
                anchor_tiles[(b, 1)] = pa1

            def anchors_max1(b):
                # top-8 of chunk 1 (DVE, emitted 2 tiles after its matmuls)
                m16 = sb_m16[:, b * 16:(b + 1) * 16]
                nc.vector.max(out=m16[:, 0:8], in_=anchor_tiles.pop((b, 1)))

            def anchors_mm2(b):
                # same-class dots chunk 2 (PE only)
                lhsT = sb_xr[:, b * 128:(b + 1) * 128]
                slab = xc_tiles[b]
                pa2 = ps.tile([128, STRIP], F32, tag="pp")
                for w in range(0, w2, 512):
                    cw = min(512, w2 - w)
                    nc.tensor.matmul(pa2[:, w:w + cw], lhsT,
                                     slab[:, w1 + w:w1 + w + cw],
                                     start=True, stop=True)
                anchor_tiles[(b, 2)] = pa2

            def anchors_max2(b):
                m16 = sb_m16[:, b * 16:(b + 1) * 16]
                nc.vector.max(out=m16[:, 8:16],
                              in_=anchor_tiles.pop((b, 2))[:, 0:w2])

            def anchors_p3(b):
                # combine the two chunk top-8s -> threshold t8[b*8+3]
                m16 = sb_m16[:, b * 16:(b + 1) * 16]
                nc.vector.max(out=sb_t8[:, b * 8:(b + 1) * 8], in_=m16)

            def count_tile(b, c):
                # threshold count over strip c for block b
                lhsT = sb_xr[:, b * 128:(b + 1) * 128]
                t_ap = sb_t8[:, b * 8 + 3: b * 8 + 4]
                cs = c * STRIP
                slot = b * NSTRIPS + c
                pt = ps.tile([128, STRIP], F32, tag="pp")
                for w in range(0, STRIP, 512):
                    nc.tensor.matmul(pt[:, w:w + 512], lhsT,
                                     sb_xt[:, cs + w:cs + w + 512],
                                     start=True, stop=True)
                if sched[slot] == "D":
                    nc.vector.tensor_scalar(pt, pt, t_ap, None,
                                            op0=ALU.is_ge, op1=ALU.add,
                                            accum_out=sb_cd[:, slot:slot + 1])
                else:
                    nc.scalar.activation(out=pt, in_=pt, func=AF.Sign,
                                         bias=t_ap, scale=-1.0,
                                         accum_out=sb_ca[:, slot:slot + 1])

            # Anchors run 2 blocks ahead of counts so the max8->threshold
            # chain never gates the count engines; anchor parts are spread
            # between count tiles so the max8 ops never form a burst in the
            # DVE queue.
            def block_step(b):
                nb = b + 2
                for c in range(NSTRIPS):
                    if nb < BLOCKS:
                        if c == 1 and nb >= XC_EARLY:
                            make_slab(nb)
                        elif c == 3:
                            anchors_mm1(nb)
                        elif c == 5:
                            anchors_max1(nb)
                        elif c == 8:
                            anchors_mm2(nb)
                        elif c == 10:
                            anchors_max2(nb)
                        elif c == 13:
                            anchors_p3(nb)
                    count_tile(b, c)

            for b in (0, 1):
                anchors_mm1(b)
                anchors_max1(b)
                anchors_mm2(b)
                anchors_max2(b)
                anchors_p3(b)
            for b in range(BLOCKS):
                block_step(b)

            nc.sync.dma_start(out=out_t[:, :], in_=sb_t8)
            nc.sync.dma_start(out=out_d[:, :], in_=sb_cd)
            nc.sync.dma_start(out=out_a[:, :], in_=sb_ca)

    nc.compile()
    return nc


def _digamma(x):
    x = np.asarray(x, np.float64).copy()
    res = np.zeros_like(x)
    for _ in range(6):
        small = x < 6.0
        if not small.any():
            break
        res[small] -= 1.0 / x[small]
        x[small] += 1.0
    inv = 1.0 / x
    inv2 = inv * inv
    res += (np.log(x) - 0.5 * inv
            - inv2 * (1.0 / 12 - inv2 * (1.0 / 120 - inv2 * (1.0 / 252 - inv2 / 240))))
    return res


def _prep(X, y):
    """Host-side normalize/sort/pad. Returns per-core input maps + row map."""
    X = np.asarray(X, np.float32)
    y = np.asarray(y).astype(np.int64)
    norms = np.maximum(np.linalg.norm(X, axis=1, keepdims=True), np.float32(EPS))
    Xn = (X / norms).astype(np.float32)
    XnT = np.ascontiguousarray(Xn.T)  # [D, N] fp32

    XnT_pad = np.zeros((D, N + 16), np.float32)
    XnT_pad[:, :N] = XnT
    counts = np.bincount(y, minlength=C)
    cls_w = max(STRIP + 128, int(-(-counts.max() // 128) * 128))
    members = [np.nonzero(y == c)[0] for c in range(C)]

    # padded sorted row layout: class-pure 128-blocks
    row_orig = np.full(NPAD, -1, np.int64)
    blk_cls = np.full(NPAD // 128, -1, np.int64)
    pos = 0
    for c in range(C):
        n_c = counts[c]
        row_orig[pos:pos + n_c] = members[c]
        nblk = -(-n_c // 128)
        blk_cls[pos // 128: pos // 128 + nblk] = c
        pos += nblk * 128
    assert pos <= NPAD

    zcol = np.zeros((D, 1), np.float32)
    iw = cls_w // 16
    in_maps = []
    for k in range(NCORES):
        rows = row_orig[k * RPC:(k + 1) * RPC]
        xr = np.where(rows[None, :] >= 0, XnT[:, np.maximum(rows, 0)], zcol)
        xr = np.ascontiguousarray(xr, dtype=np.float32)
        xc = np.zeros((D, XC_EARLY * cls_w), np.float32)
        xi = (np.zeros((128, (BLOCKS - XC_EARLY) * iw), np.uint16)
              if BLOCKS > XC_EARLY else None)
        for b in range(BLOCKS):
            c = blk_cls[(k * RPC) // 128 + b]
            mem = members[c] if c >= 0 else np.empty(0, np.int64)
            if b < XC_EARLY:
                xc[:, b * cls_w: b * cls_w + len(mem)] = XnT[:, mem]
            else:
                # wrapped uint16 indices; pad points at the zero columns at N
                ids = np.full(cls_w, N, np.uint16)
                ids[:len(mem)] = mem.astype(np.uint16)
                wrap = np.zeros((16, iw), np.uint16)
                pos = np.arange(cls_w)
                wrap[pos % 16, pos // 16] = ids
                xi[:, (b - XC_EARLY) * iw:(b - XC_EARLY + 1) * iw] = np.tile(wrap, (8, 1))
        im = {"xt": XnT_pad, "xr": xr, "xc": xc}
        if xi is not None:
            im["xi"] = xi
        in_maps.append(im)
    return in_maps, row_orig, counts, cls_w


def _m_counts_cpu(X, y):
    """Exact fp32 replication of the reference's m_i computation, blocked."""
    X = np.asarray(X, np.float32)
    y = np.asarray(y).astype(np.int64)
    norms = np.maximum(np.linalg.norm(X, axis=1, keepdims=True), np.float32(EPS))
    Xn = (X / norms).astype(np.float32)
    m = np.empty(N, np.int64)
    B = 2048
    for s in range(0, N, B):
        e = s + B
        d = (np.float32(1.0) - Xn[s:e] @ Xn.T).astype(np.float32)
        idx = np.arange(s, e)
        d[np.arange(e - s), idx] = 0.0
        ds = np.where(y[s:e, None] == y[None, :], d, np.float32(1.0e7))
        anchor = np.partition(ds, K, axis=1)[:, K]
        m[s:e] = (d <= anchor[:, None]).sum(axis=1) - 1
    return m, np.bincount(y, minlength=C)


def _combine_counts(results, row_orig):
    """Device outputs -> per-point neighbor counts m."""
    sched = _phase_b_schedule()
    is_act = np.array([s == "A" for s in sched])
    nact = is_act.reshape(BLOCKS, NSTRIPS).sum(axis=1)

    m = np.full(N, -1, np.int64)
    for k in range(NCORES):
        o_d = np.asarray(results[k]["out_d"], np.float64)
        o_a = np.asarray(results[k]["out_a"], np.float64)
        for b in range(BLOCKS):
            rows = row_orig[k * RPC + b * 128: k * RPC + (b + 1) * 128]
            sl = slice(b * NSTRIPS, (b + 1) * NSTRIPS)
            tot = (o_d[:, sl].sum(axis=1)
                   + (nact[b] * STRIP - o_a[:, sl].sum(axis=1)) / 2.0)
            cnt = np.ceil(tot - 1e-9).astype(np.int64)  # anchor half-count -> up
            valid = rows >= 0
            m[rows[valid]] = cnt[valid] - 1  # minus self
    assert (m >= 1).all(), f"bad counts: min {m.min()}"
    return m


def _m_counts_device(X, y):
    from concourse.bass_utils import run_bass_kernel_spmd

    in_maps, row_orig, counts, cls_w = _prep(X, y)
    key = ("prog", cls_w)
    if key not in _CACHE:
        _CACHE[key] = _build_program(cls_w)
    res = run_bass_kernel_spmd(_CACHE[key], in_maps, core_ids=list(range(NCORES)))
    return _combine_counts(res.results, row_orig), counts


def kernel(X, y):
    import os
    m = None
    if os.environ.get("CLUSTERMI_SKIP_DEVICE") != "1":
        try:
            m, counts = _m_counts_device(X, y)
        except Exception as e:
            if os.environ.get("CLUSTERMI_NO_FALLBACK") == "1":
                raise
            sys.stderr.write(f"clustermi: device path failed ({e}); CPU fallback\n")
    if m is None:
        m, counts = _m_counts_cpu(X, y)

    avg_m = _digamma(m.astype(np.float64)).mean()
    counts_f = counts.astype(np.float64)
    avg_N_x = float((counts_f / N * _digamma(counts_f)).sum())
    mi = float(_digamma(np.float64(N)) - avg_N_x + _digamma(np.float64(K)) - avg_m)
    return np.float32(mi / np.log(2.0))


# revision 29
# speedup vs baseline: 2.7772x; 1.2214x over previous
"""ClusterMI kernel for 8 Trainium2 NeuronCores.

Row-sharded KNN-MI:
- Host: normalize X rows (fp32, matching reference), transpose to [D=128, N]
  fp32. Sort rows by class; pad each class to a multiple of 128 rows so each
  core owns 17 class-pure 128-row blocks (8*17*128 = 17408 padded rows).
  Columns stay in ORIGINAL order (16384 wide, no padding).
- Device (per core, SPMD), per block b (interleaved phases, lhsT stationary
  across the whole block):
  Anchors: matmul block rows against a pre-gathered zero-padded [D, CLS_W]
  slab of the block's class (fp32r: full-rate PE, ~2^-12 relative noise
  vs bf16's 2^-8, keeping threshold count flips ~zero) -> PSUM fp32 dots in
  two <=1024 chunks; DVE max8 on each chunk directly from PSUM + combining
  max8 -> top-8 same-class dots; the 4th largest is the anchor threshold t
  (dot of the K=3rd same-class neighbor; d = 1 - dot is monotone decreasing
  in dot; self-dot ~1 occupies rank 1). The xc slabs live in a 6-deep
  rotating SBUF window (fp32 slabs are too big to keep all 17 resident).
  Counts: stream all 16384 columns in 1024-wide strips; per strip one engine
  counts dots >= t: DVE tensor_scalar(is_ge, add, accum_out) [exact] or ACT
  activation(Sign, scale=-1, bias=t, accum_out) [sign-sum; host converts;
  anchor-equality half-count fixed by ceil]. Strips are split DVE:ACT =
  124:148 per engine cycle costs (DVE 0.96 GHz + PSUM access penalty vs
  ACT 1.2 GHz).
- Host: m_i = (count >= t) - 1 (self), digamma in float64, assemble the
  scalar exactly like the reference formula.

Built as a Bacc module: bacc.compile() runs generate_event_semaphores which
legalizes multi-wait instructions for CoreV3 walrus (at most 1 sync wait per
instruction), so no manual observer/prime tricks are needed.
"""

import sys
import numpy as np

sys.path.insert(0, "/opt/trn_rl_repo")

N = 16384
D = 128
C = 10
K = 3
EPS = 1e-8
NCORES = 8
BLOCKS = 17                      # 128-row blocks per core
RPC = BLOCKS * 128               # rows per core (padded)
NPAD = NCORES * RPC              # 17408 padded rows
STRIP = 1024                     # count strip width
NSTRIPS = N // STRIP             # 16 strips per block
XC_BUFS = 9                      # rotating xc slab window depth
XC_EARLY = BLOCKS               # all slabs DMAd from host (on-device GpSimd
                                 # gather measured 20x slower than DMA)

_CACHE = {}


N_DVE_TILES = 119   # DVE tensor_scalar counts; ACT takes the rest


def _phase_b_schedule():
    """Deterministic DVE/ACT assignment for the 17*16 count tiles.

    Measured per-tile HW costs: DVE 1430 ns (tensor_scalar + accumulator
    read + PSUM access), ACT 1382 ns (activate + accumulator read). DVE also
    carries the 17 per-block max8 chains (~2.4 us each):
    solve 40.2us + 1430*x = 1382*(272-x) -> x = 119.
    """
    total = BLOCKS * NSTRIPS
    n_dve = N_DVE_TILES
    sched = []
    acc = 0
    for _ in range(total):
        acc += n_dve
        if acc >= total:
            acc -= total
            sched.append("D")
        else:
            sched.append("A")
    assert sched.count("D") == n_dve
    return sched


def _build_program(cls_w):
    import concourse.tile as tile
    from concourse import mybir
    from concourse.bacc import Bacc
    from contextlib import ExitStack

    F32 = mybir.dt.float32
    F32R = mybir.dt.float32r
    U16 = mybir.dt.uint16
    AF = mybir.ActivationFunctionType
    ALU = mybir.AluOpType

    assert cls_w % 128 == 0 and STRIP < cls_w <= 2 * STRIP
    iw = cls_w // 16              # wrapped index columns per slab

    nc = Bacc("TRN2", target_bir_lowering=False)
    # xt carries 16 zero pad columns at the end: gather pad-indices point there
    xt = nc.dram_tensor("xt", [D, N + 16], F32R, kind="ExternalInput")
    xr = nc.dram_tensor("xr", [D, RPC], F32R, kind="ExternalInput")
    xc = nc.dram_tensor("xc", [D, XC_EARLY * cls_w], F32R, kind="ExternalInput")
    xi = (nc.dram_tensor("xi", [128, (BLOCKS - XC_EARLY) * iw], U16,
                         kind="ExternalInput")
          if BLOCKS > XC_EARLY else None)
    out_t = nc.dram_tensor("out_t", [128, BLOCKS * 8], F32, kind="ExternalOutput")
    out_d = nc.dram_tensor("out_d", [128, BLOCKS * NSTRIPS], F32, kind="ExternalOutput")
    out_a = nc.dram_tensor("out_a", [128, BLOCKS * NSTRIPS], F32, kind="ExternalOutput")

    sched = _phase_b_schedule()
    w1 = STRIP                # first anchor chunk width
    w2 = cls_w - STRIP        # second anchor chunk width (<= 1024)

    with tile.TileContext(nc) as tc:
        with ExitStack() as ctx:
            singles = ctx.enter_context(tc.tile_pool(name="singles", bufs=1))
            xcp = ctx.enter_context(tc.tile_pool(name="xcp", bufs=XC_BUFS))
            gtmp = ctx.enter_context(tc.tile_pool(name="gtmp", bufs=2))
            ps = ctx.enter_context(tc.tile_pool(name="ps", bufs=4, space="PSUM"))

            sb_xr = singles.tile([D, RPC], F32R)
            nc.sync.dma_start(out=sb_xr, in_=xr[:, :])
            sb_xt = singles.tile([D, N + 16], F32R)
            nc.sync.dma_start(out=sb_xt[:, N:N + 16], in_=xt[:, N:N + 16])
            if xi is not None:
                sb_xi = singles.tile([128, (BLOCKS - XC_EARLY) * iw], U16)
                nc.sync.dma_start(out=sb_xi, in_=xi[:, :])

            # Interleave xc slab / xt strip DMAs so early blocks' slabs and
            # early strips arrive first; slabs rotate through XC_BUFS slots.
            # Slabs for blocks >= XC_EARLY are gathered from sb_xt on GpSimd
            # once all xt strips have landed (emitted lazily below).
            dma_slabs = []
            xt_loaded = 0
            for b in range(XC_EARLY):
                t_xc = xcp.tile([D, cls_w], F32R, tag="xc")
                s = b * cls_w
                h = cls_w // 2
                nc.sync.dma_start(out=t_xc[:, 0:h], in_=xc[:, s:s + h])
                nc.sync.dma_start(out=t_xc[:, h:cls_w], in_=xc[:, s + h:s + cls_w])
                dma_slabs.append(t_xc)
                if xt_loaded < NSTRIPS:
                    for c in range(xt_loaded, min(xt_loaded + 4, NSTRIPS)):
                        cs = c * STRIP
                        nc.sync.dma_start(out=sb_xt[:, cs:cs + STRIP],
                                          in_=xt[:, cs:cs + STRIP])
                    xt_loaded = min(xt_loaded + 4, NSTRIPS)
            for c in range(xt_loaded, NSTRIPS):
                cs = c * STRIP
                nc.sync.dma_start(out=sb_xt[:, cs:cs + STRIP],
                                  in_=xt[:, cs:cs + STRIP])

            xc_tiles = {}
            for b in range(XC_EARLY):
                xc_tiles[b] = dma_slabs[b]

            def make_slab(b):
                # gather block b's class columns from sb_xt (GpSimd), then
                # bounce SBUF->SBUF through DMA: fp32r matmuls only accept
                # DMA-produced (or pre-rounded) inputs.
                tmp = gtmp.tile([D, cls_w], F32, tag="gt")
                ib = (b - XC_EARLY) * iw
                for gs in range(0, cls_w, STRIP):
                    gw = min(STRIP, cls_w - gs)
                    nc.gpsimd.indirect_copy(
                        tmp[:, gs:gs + gw],
                        sb_xt[:, 0:N + 16].bitcast(F32),
                        sb_xi[:, ib + gs // 16: ib + (gs + gw) // 16], True)
                t_xc = xcp.tile([D, cls_w], F32R, tag="xc")
                nc.sync.dma_start(out=t_xc, in_=tmp.bitcast(F32R))
                xc_tiles[b] = t_xc

            sb_t8 = singles.tile([128, BLOCKS * 8], F32)
            sb_m16 = singles.tile([128, BLOCKS * 16], F32)
            sb_cd = singles.tile([128, BLOCKS * NSTRIPS], F32)
            sb_ca = singles.tile([128, BLOCKS * NSTRIPS], F32)
            nc.gpsimd.memset(sb_cd, 0.0)
            nc.gpsimd.memset(sb_ca, 0.0)

            anchor_tiles = {}

            def anchors_mm1(b):
                # same-class dots chunk 1 (PE only)
                lhsT = sb_xr[:, b * 128:(b + 1) * 128]
                slab = xc_tiles[b]
                pa1 = ps.tile([128, STRIP], F32, tag="pp")
                for w in range(0, w1, 512):
                    cw = min(512, w1 - w)
                    nc.tensor.matmul(pa1[:, w:w + cw], lhsT, slab[:, w:w + cw],
                                     start=True, stop=True)
                anchor_tiles[(b, 1)] = pa1

            def anchors_max1(b):
                # top-8 of chunk 1 (DVE, emitted 2 tiles after its matmuls)
                m16 = sb_m16[:, b * 16:(b + 1) * 16]
                nc.vector.max(out=m16[:, 0:8], in_=anchor_tiles.pop((b, 1)))

            def anchors_mm2(b):
                # same-class dots chunk 2 (PE only)
                lhsT = sb_xr[:, b * 128:(b + 1) * 128]
                slab = xc_tiles[b]
                pa2 = ps.tile([128, STRIP], F32, tag="pp")
                for w in range(0, w2, 512):
                    cw = min(512, w2 - w)
                    nc.tensor.matmul(pa2[:, w:w + cw], lhsT,
                                     slab[:, w1 + w:w1 + w + cw],
                                     start=True, stop=True)
                anchor_tiles[(b, 2)] = pa2

            def anchors_max2(b):
                m16 = sb_m16[:, b * 16:(b + 1) * 16]
                nc.vector.max(out=m16[:, 8:16],
                              in_=anchor_tiles.pop((b, 2))[:, 0:w2])

            def anchors_p3(b):
                # combine the two chunk top-8s -> threshold t8[b*8+3]
                m16 = sb_m16[:, b * 16:(b + 1) * 16]
                nc.vector.max(out=sb_t8[:, b * 8:(b + 1) * 8], in_=m16)

            def count_tile(b, c):
                # threshold count over strip c for block b
                lhsT = sb_xr[:, b * 128:(b + 1) * 128]
                t_ap = sb_t8[:, b * 8 + 3: b * 8 + 4]
                cs = c * STRIP
                slot = b * NSTRIPS + c
                pt = ps.tile([128, STRIP], F32, tag="pp")
                for w in range(0, STRIP, 512):
                    nc.tensor.matmul(pt[:, w:w + 512], lhsT,
                                     sb_xt[:, cs + w:cs + w + 512],
                                     start=True, stop=True)
                if sched[slot] == "D":
                    nc.vector.tensor_scalar(pt, pt, t_ap, None,
                                            op0=ALU.is_ge, op1=ALU.add,
                                            accum_out=sb_cd[:, slot:slot + 1])
                else:
                    nc.scalar.activation(out=pt, in_=pt, func=AF.Sign,
                                         bias=t_ap, scale=-1.0,
                                         accum_out=sb_ca[:, slot:slot + 1])

            # Anchors run 2 blocks ahead of counts so the max8->threshold
            # chain never gates the count engines; anchor parts are spread
            # between count tiles so the max8 ops never form a burst in the
            # DVE queue.
            def block_step(b):
                nb = b + 2
                for c in range(NSTRIPS):
                    if nb < BLOCKS:
                        if c == 3:
                            anchors_mm1(nb)
                            anchors_max1(nb)
                        elif c == 9:
                            anchors_mm2(nb)
                            anchors_max2(nb)
                        elif c == 13:
                            anchors_p3(nb)
                    count_tile(b, c)

            for b in (0, 1):
                anchors_mm1(b)
                anchors_max1(b)
                anchors_mm2(b)
                anchors_max2(b)
                anchors_p3(b)
            for b in range(BLOCKS):
                block_step(b)

            nc.sync.dma_start(out=out_t[:, :], in_=sb_t8)
            nc.sync.dma_start(out=out_d[:, :], in_=sb_cd)
            nc.sync.dma_start(out=out_a[:, :], in_=sb_ca)

    nc.compile()
    return nc


def _digamma(x):
    x = np.asarray(x, np.float64).copy()
    res = np.zeros_like(x)
    for _ in range(6):
        small = x < 6.0
        if not small.any():
            break
        res[small] -= 1.0 / x[small]
        x[small] += 1.0
    inv = 1.0 / x
    inv2 = inv * inv
    res += (np.log(x) - 0.5 * inv
            - inv2 * (1.0 / 12 - inv2 * (1.0 / 120 - inv2 * (1.0 / 252 - inv2 / 240))))
    return res


def _prep(X, y):
    """Host-side normalize/sort/pad. Returns per-core input maps + row map."""
    X = np.asarray(X, np.float32)
    y = np.asarray(y).astype(np.int64)
    norms = np.maximum(np.linalg.norm(X, axis=1, keepdims=True), np.float32(EPS))
    Xn = (X / norms).astype(np.float32)
    XnT = np.ascontiguousarray(Xn.T)  # [D, N] fp32

    XnT_pad = np.zeros((D, N + 16), np.float32)
    XnT_pad[:, :N] = XnT
    counts = np.bincount(y, minlength=C)
    cls_w = max(STRIP + 128, int(-(-counts.max() // 128) * 128))
    members = [np.nonzero(y == c)[0] for c in range(C)]

    # padded sorted row layout: class-pure 128-blocks
    row_orig = np.full(NPAD, -1, np.int64)
    blk_cls = np.full(NPAD // 128, -1, np.int64)
    pos = 0
    for c in range(C):
        n_c = counts[c]
        row_orig[pos:pos + n_c] = members[c]
        nblk = -(-n_c // 128)
        blk_cls[pos // 128: pos // 128 + nblk] = c
        pos += nblk * 128
    assert pos <= NPAD

    zcol = np.zeros((D, 1), np.float32)
    iw = cls_w // 16
    in_maps = []
    for k in range(NCORES):
        rows = row_orig[k * RPC:(k + 1) * RPC]
        xr = np.where(rows[None, :] >= 0, XnT[:, np.maximum(rows, 0)], zcol)
        xr = np.ascontiguousarray(xr, dtype=np.float32)
        xc = np.zeros((D, XC_EARLY * cls_w), np.float32)
        xi = (np.zeros((128, (BLOCKS - XC_EARLY) * iw), np.uint16)
              if BLOCKS > XC_EARLY else None)
        for b in range(BLOCKS):
            c = blk_cls[(k * RPC) // 128 + b]
            mem = members[c] if c >= 0 else np.empty(0, np.int64)
            if b < XC_EARLY:
                xc[:, b * cls_w: b * cls_w + len(mem)] = XnT[:, mem]
            else:
                # wrapped uint16 indices; pad points at the zero columns at N
                ids = np.full(cls_w, N, np.uint16)
                ids[:len(mem)] = mem.astype(np.uint16)
                wrap = np.zeros((16, iw), np.uint16)
                pos = np.arange(cls_w)
                wrap[pos % 16, pos // 16] = ids
                xi[:, (b - XC_EARLY) * iw:(b - XC_EARLY + 1) * iw] = np.tile(wrap, (8, 1))
        im = {"xt": XnT_pad, "xr": xr, "xc": xc}
        if xi is not None:
            im["xi"] = xi
        in_maps.append(im)
    return in_maps, row_orig, counts, cls_w


def _m_counts_cpu(X, y):
    """Exact fp32 replication of the reference's m_i computation, blocked."""
    X = np.asarray(X, np.float32)
    y = np.asarray(y).astype(np.int64)
    norms = np.maximum(np.linalg.norm(X, axis=1, keepdims=True), np.float32(EPS))
    Xn = (X / norms).astype(np.float32)
    m = np.empty(N, np.int64)
    B = 2048
    for s in range(0, N, B):
        e = s + B
        d = (np.float32(1.0) - Xn[s:e] @ Xn.T).astype(np.float32)
        idx = np.arange(s, e)
        d[np.arange(e - s), idx] = 0.0
        ds = np.where(y[s:e, None] == y[None, :], d, np.float32(1.0e7))
        anchor = np.partition(ds, K, axis=1)[:, K]
        m[s:e] = (d <= anchor[:, None]).sum(axis=1) - 1
    return m, np.bincount(y, minlength=C)


def _combine_counts(results, row_orig):
    """Device outputs -> per-point neighbor counts m."""
    sched = _phase_b_schedule()
    is_act = np.array([s == "A" for s in sched])
    nact = is_act.reshape(BLOCKS, NSTRIPS).sum(axis=1)

    m = np.full(N, -1, np.int64)
    for k in range(NCORES):
        o_d = np.asarray(results[k]["out_d"], np.float64)
        o_a = np.asarray(results[k]["out_a"], np.float64)
        for b in range(BLOCKS):
            rows = row_orig[k * RPC + b * 128: k * RPC + (b + 1) * 128]
            sl = slice(b * NSTRIPS, (b + 1) * NSTRIPS)
            tot = (o_d[:, sl].sum(axis=1)
                   + (nact[b] * STRIP - o_a[:, sl].sum(axis=1)) / 2.0)
            cnt = np.ceil(tot - 1e-9).astype(np.int64)  # anchor half-count -> up
            valid = rows >= 0
            m[rows[valid]] = cnt[valid] - 1  # minus self
    assert (m >= 1).all(), f"bad counts: min {m.min()}"
    return m


def _m_counts_device(X, y):
    from concourse.bass_utils import run_bass_kernel_spmd

    in_maps, row_orig, counts, cls_w = _prep(X, y)
    key = ("prog", cls_w)
    if key not in _CACHE:
        _CACHE[key] = _build_program(cls_w)
    res = run_bass_kernel_spmd(_CACHE[key], in_maps, core_ids=list(range(NCORES)))
    return _combine_counts(res.results, row_orig), counts


def kernel(X, y):
    import os
    m = None
    if os.environ.get("CLUSTERMI_SKIP_DEVICE") != "1":
        try:
            m, counts = _m_counts_device(X, y)
        except Exception as e:
            if os.environ.get("CLUSTERMI_NO_FALLBACK") == "1":
                raise
            sys.stderr.write(f"clustermi: device path failed ({e}); CPU fallback\n")
    if m is None:
        m, counts = _m_counts_cpu(X, y)

    avg_m = _digamma(m.astype(np.float64)).mean()
    counts_f = counts.astype(np.float64)
    avg_N_x = float((counts_f / N * _digamma(counts_f)).sum())
    mi = float(_digamma(np.float64(N)) - avg_N_x + _digamma(np.float64(K)) - avg_m)
    return np.float32(mi / np.log(2.0))


# revision 35
# speedup vs baseline: 2.8156x; 1.0138x over previous
"""ClusterMI kernel for 8 Trainium2 NeuronCores.

Row-sharded KNN-MI:
- Host: normalize X rows (fp32, matching reference), transpose to [D=128, N]
  fp32. Sort rows by class; pad each class to a multiple of 128 rows so each
  core owns 17 class-pure 128-row blocks (8*17*128 = 17408 padded rows).
  Columns stay in ORIGINAL order (16384 wide, no padding).
- Device (per core, SPMD), per block b (interleaved phases, lhsT stationary
  across the whole block):
  Anchors: matmul block rows against a pre-gathered zero-padded [D, CLS_W]
  slab of the block's class (fp32r: full-rate PE, ~2^-12 relative noise
  vs bf16's 2^-8, keeping threshold count flips ~zero) -> PSUM fp32 dots in
  two <=1024 chunks; DVE max8 on each chunk directly from PSUM + combining
  max8 -> top-8 same-class dots; the 4th largest is the anchor threshold t
  (dot of the K=3rd same-class neighbor; d = 1 - dot is monotone decreasing
  in dot; self-dot ~1 occupies rank 1). The xc slabs live in a 6-deep
  rotating SBUF window (fp32 slabs are too big to keep all 17 resident).
  Counts: stream all 16384 columns in 1024-wide strips; per strip one engine
  counts dots >= t: DVE tensor_scalar(is_ge, add, accum_out) [exact] or ACT
  activation(Sign, scale=-1, bias=t, accum_out) [sign-sum; host converts;
  anchor-equality half-count fixed by ceil]. Strips are split DVE:ACT =
  124:148 per engine cycle costs (DVE 0.96 GHz + PSUM access penalty vs
  ACT 1.2 GHz).
- Host: m_i = (count >= t) - 1 (self), digamma in float64, assemble the
  scalar exactly like the reference formula.

Built as a Bacc module: bacc.compile() runs generate_event_semaphores which
legalizes multi-wait instructions for CoreV3 walrus (at most 1 sync wait per
instruction), so no manual observer/prime tricks are needed.
"""

import sys
import numpy as np

sys.path.insert(0, "/opt/trn_rl_repo")

N = 16384
D = 128
C = 10
K = 3
EPS = 1e-8
NCORES = 8
BLOCKS = 17                      # 128-row blocks per core
RPC = BLOCKS * 128               # rows per core (padded)
NPAD = NCORES * RPC              # 17408 padded rows
STRIP = 1024                     # count strip width
NSTRIPS = N // STRIP             # 16 strips per block
XC_BUFS = 9                      # rotating xc slab window depth
XC_EARLY = BLOCKS               # all slabs DMAd from host (on-device GpSimd
                                 # gather measured 20x slower than DMA)

_CACHE = {}


N_DVE_TILES = 119   # DVE tensor_scalar counts; ACT takes the rest


def _phase_b_schedule():
    """Deterministic DVE/ACT assignment for the 17*16 count tiles.

    Measured per-tile HW costs: DVE 1430 ns (tensor_scalar + accumulator
    read + PSUM access), ACT 1382 ns (activate + accumulator read). DVE also
    carries the 17 per-block max8 chains (~2.4 us each):
    solve 40.2us + 1430*x = 1382*(272-x) -> x = 119.
    """
    total = BLOCKS * NSTRIPS
    n_dve = N_DVE_TILES
    sched = []
    acc = 0
    for _ in range(total):
        acc += n_dve
        if acc >= total:
            acc -= total
            sched.append("D")
        else:
            sched.append("A")
    assert sched.count("D") == n_dve
    return sched


def _build_program(cls_w):
    import concourse.tile as tile
    from concourse import mybir
    from concourse.bacc import Bacc
    from contextlib import ExitStack

    F32 = mybir.dt.float32
    F32R = mybir.dt.float32r
    U16 = mybir.dt.uint16
    AF = mybir.ActivationFunctionType
    ALU = mybir.AluOpType

    assert cls_w % 128 == 0 and STRIP < cls_w <= 2 * STRIP
    iw = cls_w // 16              # wrapped index columns per slab

    nc = Bacc("TRN2", target_bir_lowering=False)
    # xt carries 16 zero pad columns at the end: gather pad-indices point there
    xt = nc.dram_tensor("xt", [D, N + 16], F32R, kind="ExternalInput")
    xr = nc.dram_tensor("xr", [D, RPC], F32R, kind="ExternalInput")
    xc = nc.dram_tensor("xc", [D, XC_EARLY * cls_w], F32R, kind="ExternalInput")
    xi = (nc.dram_tensor("xi", [128, (BLOCKS - XC_EARLY) * iw], U16,
                         kind="ExternalInput")
          if BLOCKS > XC_EARLY else None)
    out_t = nc.dram_tensor("out_t", [128, BLOCKS * 8], F32, kind="ExternalOutput")
    out_d = nc.dram_tensor("out_d", [128, BLOCKS * NSTRIPS], F32, kind="ExternalOutput")
    out_a = nc.dram_tensor("out_a", [128, BLOCKS * NSTRIPS], F32, kind="ExternalOutput")

    sched = _phase_b_schedule()
    w1 = STRIP                # first anchor chunk width
    w2 = cls_w - STRIP        # second anchor chunk width (<= 1024)

    with tile.TileContext(nc) as tc:
        with ExitStack() as ctx:
            singles = ctx.enter_context(tc.tile_pool(name="singles", bufs=1))
            ps = ctx.enter_context(tc.tile_pool(name="ps", bufs=4, space="PSUM"))

            sb_xr = singles.tile([D, RPC], F32R)
            sb_xt = singles.tile([D, N + 16], F32R)
            sb_xc = singles.tile([D, BLOCKS * cls_w], F32R)

            # Fine-grained interleaved input DMAs: slab halves + 4 xt strips
            # per slab group, so early blocks' slabs and all xt strips land
            # fast (xt is fully needed by block 1; slab b only by block b-2).
            nc.sync.dma_start(out=sb_xr, in_=xr[:, :])
            xt_loaded = 0
            for b in range(BLOCKS):
                s = b * cls_w
                h = min(STRIP, cls_w)  # align split to the anchor chunks
                nc.sync.dma_start(out=sb_xc[:, s:s + h], in_=xc[:, s:s + h])
                nc.sync.dma_start(out=sb_xc[:, s + h:s + cls_w], in_=xc[:, s + h:s + cls_w])
                if xt_loaded < NSTRIPS:
                    for c in range(xt_loaded, min(xt_loaded + 4, NSTRIPS)):
                        cs = c * STRIP
                        nc.sync.dma_start(out=sb_xt[:, cs:cs + STRIP],
                                          in_=xt[:, cs:cs + STRIP])
                    xt_loaded = min(xt_loaded + 4, NSTRIPS)
            nc.sync.dma_start(out=sb_xt[:, N:N + 16], in_=xt[:, N:N + 16])

            xc_tiles = {b: sb_xc[:, b * cls_w:(b + 1) * cls_w] for b in range(BLOCKS)}

            sb_t8 = singles.tile([128, BLOCKS * 8], F32)
            sb_m16 = singles.tile([128, BLOCKS * 16], F32)
            sb_cd = singles.tile([128, BLOCKS * NSTRIPS], F32)
            sb_ca = singles.tile([128, BLOCKS * NSTRIPS], F32)
            nc.gpsimd.memset(sb_cd, 0.0)
            nc.gpsimd.memset(sb_ca, 0.0)

            anchor_tiles = {}

            def anchors_mm1(b):
                # same-class dots chunk 1 (PE only)
                lhsT = sb_xr[:, b * 128:(b + 1) * 128]
                slab = xc_tiles[b]
                pa1 = ps.tile([128, STRIP], F32, tag="pp")
                for w in range(0, w1, 512):
                    cw = min(512, w1 - w)
                    nc.tensor.matmul(pa1[:, w:w + cw], lhsT, slab[:, w:w + cw],
                                     start=True, stop=True)
                anchor_tiles[(b, 1)] = pa1

            def anchors_max1(b):
                # top-8 of chunk 1 (DVE, emitted 2 tiles after its matmuls)
                m16 = sb_m16[:, b * 16:(b + 1) * 16]
                nc.vector.max(out=m16[:, 0:8], in_=anchor_tiles.pop((b, 1)))

            def anchors_mm2(b):
                # same-class dots chunk 2 (PE only)
                lhsT = sb_xr[:, b * 128:(b + 1) * 128]
                slab = xc_tiles[b]
                pa2 = ps.tile([128, STRIP], F32, tag="pp")
                for w in range(0, w2, 512):
                    cw = min(512, w2 - w)
                    nc.tensor.matmul(pa2[:, w:w + cw], lhsT,
                                     slab[:, w1 + w:w1 + w + cw],
                                     start=True, stop=True)
                anchor_tiles[(b, 2)] = pa2

            def anchors_max2(b):
                m16 = sb_m16[:, b * 16:(b + 1) * 16]
                nc.vector.max(out=m16[:, 8:16],
                              in_=anchor_tiles.pop((b, 2))[:, 0:w2])

            def anchors_p3(b):
                # combine the two chunk top-8s -> threshold t8[b*8+3]
                m16 = sb_m16[:, b * 16:(b + 1) * 16]
                nc.vector.max(out=sb_t8[:, b * 8:(b + 1) * 8], in_=m16)

            def count_tile(b, c):
                # threshold count over strip c for block b
                lhsT = sb_xr[:, b * 128:(b + 1) * 128]
                t_ap = sb_t8[:, b * 8 + 3: b * 8 + 4]
                cs = c * STRIP
                slot = b * NSTRIPS + c
                pt = ps.tile([128, STRIP], F32, tag="pp")
                for w in range(0, STRIP, 512):
                    nc.tensor.matmul(pt[:, w:w + 512], lhsT,
                                     sb_xt[:, cs + w:cs + w + 512],
                                     start=True, stop=True)
                if sched[slot] == "D":
                    nc.vector.tensor_scalar(pt, pt, t_ap, None,
                                            op0=ALU.is_ge, op1=ALU.add,
                                            accum_out=sb_cd[:, slot:slot + 1])
                else:
                    nc.scalar.activation(out=pt, in_=pt, func=AF.Sign,
                                         bias=t_ap, scale=-1.0,
                                         accum_out=sb_ca[:, slot:slot + 1])

            # Anchors run 2 blocks ahead of counts so the max8->threshold
            # chain never gates the count engines; anchor parts are spread
            # between count tiles so the max8 ops never form a burst in the
            # DVE queue.
            def block_step(b):
                nb = b + 2
                for c in range(NSTRIPS):
                    if nb < BLOCKS:
                        if c == 3:
                            anchors_mm1(nb)
                            anchors_max1(nb)
                        elif c == 9:
                            anchors_mm2(nb)
                            anchors_max2(nb)
                        elif c == 13:
                            anchors_p3(nb)
                    count_tile(b, c)

            for b in (0, 1):
                anchors_mm1(b)
                anchors_max1(b)
                anchors_mm2(b)
                anchors_max2(b)
                anchors_p3(b)
            for b in range(BLOCKS):
                block_step(b)

            nc.sync.dma_start(out=out_t[:, :], in_=sb_t8)
            nc.sync.dma_start(out=out_d[:, :], in_=sb_cd)
            nc.sync.dma_start(out=out_a[:, :], in_=sb_ca)

    nc.compile()
    return nc


def _digamma(x):
    x = np.asarray(x, np.float64).copy()
    res = np.zeros_like(x)
    for _ in range(6):
        small = x < 6.0
        if not small.any():
            break
        res[small] -= 1.0 / x[small]
        x[small] += 1.0
    inv = 1.0 / x
    inv2 = inv * inv
    res += (np.log(x) - 0.5 * inv
            - inv2 * (1.0 / 12 - inv2 * (1.0 / 120 - inv2 * (1.0 / 252 - inv2 / 240))))
    return res


def _prep(X, y):
    """Host-side normalize/sort/pad. Returns per-core input maps + row map."""
    X = np.asarray(X, np.float32)
    y = np.asarray(y).astype(np.int64)
    norms = np.maximum(np.linalg.norm(X, axis=1, keepdims=True), np.float32(EPS))
    Xn = (X / norms).astype(np.float32)
    XnT = np.ascontiguousarray(Xn.T)  # [D, N] fp32

    XnT_pad = np.zeros((D, N + 16), np.float32)
    XnT_pad[:, :N] = XnT
    counts = np.bincount(y, minlength=C)
    cls_w = max(STRIP + 128, int(-(-counts.max() // 128) * 128))
    members = [np.nonzero(y == c)[0] for c in range(C)]

    # padded sorted row layout: class-pure 128-blocks
    row_orig = np.full(NPAD, -1, np.int64)
    blk_cls = np.full(NPAD // 128, -1, np.int64)
    pos = 0
    for c in range(C):
        n_c = counts[c]
        row_orig[pos:pos + n_c] = members[c]
        nblk = -(-n_c // 128)
        blk_cls[pos // 128: pos // 128 + nblk] = c
        pos += nblk * 128
    assert pos <= NPAD

    zcol = np.zeros((D, 1), np.float32)
    iw = cls_w // 16
    in_maps = []
    for k in range(NCORES):
        rows = row_orig[k * RPC:(k + 1) * RPC]
        xr = np.where(rows[None, :] >= 0, XnT[:, np.maximum(rows, 0)], zcol)
        xr = np.ascontiguousarray(xr, dtype=np.float32)
        xc = np.zeros((D, XC_EARLY * cls_w), np.float32)
        xi = (np.zeros((128, (BLOCKS - XC_EARLY) * iw), np.uint16)
              if BLOCKS > XC_EARLY else None)
        for b in range(BLOCKS):
            c = blk_cls[(k * RPC) // 128 + b]
            mem = members[c] if c >= 0 else np.empty(0, np.int64)
            if b < XC_EARLY:
                xc[:, b * cls_w: b * cls_w + len(mem)] = XnT[:, mem]
            else:
                # wrapped uint16 indices; pad points at the zero columns at N
                ids = np.full(cls_w, N, np.uint16)
                ids[:len(mem)] = mem.astype(np.uint16)
                wrap = np.zeros((16, iw), np.uint16)
                pos = np.arange(cls_w)
                wrap[pos % 16, pos // 16] = ids
                xi[:, (b - XC_EARLY) * iw:(b - XC_EARLY + 1) * iw] = np.tile(wrap, (8, 1))
        im = {"xt": XnT_pad, "xr": xr, "xc": xc}
        if xi is not None:
            im["xi"] = xi
        in_maps.append(im)
    return in_maps, row_orig, counts, cls_w


def _m_counts_cpu(X, y):
    """Exact fp32 replication of the reference's m_i computation, blocked."""
    X = np.asarray(X, np.float32)
    y = np.asarray(y).astype(np.int64)
    norms = np.maximum(np.linalg.norm(X, axis=1, keepdims=True), np.float32(EPS))
    Xn = (X / norms).astype(np.float32)
    m = np.empty(N, np.int64)
    B = 2048
    for s in range(0, N, B):
        e = s + B
        d = (np.float32(1.0) - Xn[s:e] @ Xn.T).astype(np.float32)
        idx = np.arange(s, e)
        d[np.arange(e - s), idx] = 0.0
        ds = np.where(y[s:e, None] == y[None, :], d, np.float32(1.0e7))
        anchor = np.partition(ds, K, axis=1)[:, K]
        m[s:e] = (d <= anchor[:, None]).sum(axis=1) - 1
    return m, np.bincount(y, minlength=C)


def _combine_counts(results, row_orig):
    """Device outputs -> per-point neighbor counts m."""
    sched = _phase_b_schedule()
    is_act = np.array([s == "A" for s in sched])
    nact = is_act.reshape(BLOCKS, NSTRIPS).sum(axis=1)

    m = np.full(N, -1, np.int64)
    for k in range(NCORES):
        o_d = np.asarray(results[k]["out_d"], np.float64)
        o_a = np.asarray(results[k]["out_a"], np.float64)
        for b in range(BLOCKS):
            rows = row_orig[k * RPC + b * 128: k * RPC + (b + 1) * 128]
            sl = slice(b * NSTRIPS, (b + 1) * NSTRIPS)
            tot = (o_d[:, sl].sum(axis=1)
                   + (nact[b] * STRIP - o_a[:, sl].sum(axis=1)) / 2.0)
            cnt = np.ceil(tot - 1e-9).astype(np.int64)  # anchor half-count -> up
            valid = rows >= 0
            m[rows[valid]] = cnt[valid] - 1  # minus self
    assert (m >= 1).all(), f"bad counts: min {m.min()}"
    return m


def _m_counts_device(X, y):
    from concourse.bass_utils import run_bass_kernel_spmd

    in_maps, row_orig, counts, cls_w = _prep(X, y)
    key = ("prog", cls_w)
    if key not in _CACHE:
        _CACHE[key] = _build_program(cls_w)
    res = run_bass_kernel_spmd(_CACHE[key], in_maps, core_ids=list(range(NCORES)))
    return _combine_counts(res.results, row_orig), counts


def kernel(X, y):
    import os
    m = None
    if os.environ.get("CLUSTERMI_SKIP_DEVICE") != "1":
        try:
            m, counts = _m_counts_device(X, y)
        except Exception as e:
            if os.environ.get("CLUSTERMI_NO_FALLBACK") == "1":
                raise
            sys.stderr.write(f"clustermi: device path failed ({e}); CPU fallback\n")
    if m is None:
        m, counts = _m_counts_cpu(X, y)

    avg_m = _digamma(m.astype(np.float64)).mean()
    counts_f = counts.astype(np.float64)
    avg_N_x = float((counts_f / N * _digamma(counts_f)).sum())
    mi = float(_digamma(np.float64(N)) - avg_N_x + _digamma(np.float64(K)) - avg_m)
    return np.float32(mi / np.log(2.0))
